# revision 30
# baseline (speedup 1.0000x reference)
"""Squeeze-and-Excitation gate kernel for Trainium2 (Bass/Tile).

Reference computation (per sample b):
    s = mean(x[b], axis=(H, W))                # [C]
    h = relu(w1 @ s + b1)                      # [Cr]
    g = sigmoid(w2 @ h + b2)                   # [C]
    out[b] = x[b] * g[:, None, None]

Sharding: data-parallel over batch across 8 NeuronCores (8 samples each),
gate weights replicated. Each core streams each sample through SBUF once
(1x HBM read + 1x write of x, the memory floor). Perf structure:
  - x streams as bf16 (host converts both ways): halves DMA bytes while
    pool/gate math stays f32; worst-case elementwise error ~1%, inside
    the 2e-2 gate;
  - half-sample tiles [128, 2*HW]: partition p holds channels 4p..4p+3
    (contiguous in DRAM), half A = channels {4p,4p+1}, half B =
    {4p+2,4p+3}; every DMA line is 12.5KB and the 16 HW queues are
    byte-rate-limited (~26.6GB/s each), so granularity buys interleave,
    not bandwidth;
  - 12 rotating half-tiles: load h reuses the buffer of half h-12, so
    late loads are paced by store completions and stores interleave with
    loads on the queues instead of piling up at the drain;
  - per half, one block reduces on DVE (reduce_sum) and one on ACT
    (Copy+accum_out into a scratch tile, so both engines only READ the
    x tile and run concurrently); scale muls all on DVE (bf16 4x mode);
  - sample 0's two halves are pinned; their stores are issued right
    before the last sample's, covering the final gate latency.
"""

import contextlib
import os
import sys
import types

import numpy as np

import concourse.bacc as bacc
import concourse.mybir as mybir
import concourse.tile as tile
from concourse import bass_utils


def _ensure_axon_hooks():
    """bass_utils imports antenv.axon_hooks when BASS_TRACE=1 under axon;
    some images lack that module. Provide it (with the ctypes NTFF hook if
    the axon .so is present) so tracing degrades gracefully instead of
    crashing. Inert when the real module exists."""
    try:
        import antenv.axon_hooks  # noqa: F401
        return
    except ImportError:
        pass
    try:
        import antenv
    except ImportError:
        return
    mod = types.ModuleType("antenv.axon_hooks")
    _state = {"h": None}
    mod.set_axon_ntff_profile_hook = lambda h: _state.__setitem__("h", h)
    mod.get_axon_ntff_profile_hook = lambda: _state.get("h")
    sys.modules["antenv.axon_hooks"] = mod
    antenv.axon_hooks = mod
    so = "/opt/axon/libaxon_pjrt.so"
    if os.path.exists(so):
        try:
            from trn_agent_boot.trn_boot import _ntff_profile_via_ctypes
            mod.set_axon_ntff_profile_hook(_ntff_profile_via_ctypes(so))
        except Exception:
            pass


_ensure_axon_hooks()

N_CORES = 8
B, C, H, W = 64, 512, 56, 56
HW = H * W              # 3136
BL = B // N_CORES       # 8 local samples per core
P = 128                 # SBUF partitions
F = C // P              # 4 channels folded per partition
FHW = F * HW            # 12544 free elems per partition
HHW = FHW // 2          # 6272: half-sample free elems (12.5KB bf16 lines)
R = 32                  # squeezed channels (Cr)
INV_HW = 1.0 / HW

_CACHE = {}
LAST_RESULTS = None     # test harness reads trace/exec info from here


def _emit(ctx, tc, out, x, w1t, b1, w2t, b2t):
    nc = tc.nc
    f32 = mybir.dt.float32
    bf16 = mybir.dt.bfloat16
    f16 = mybir.dt.float16

    singles = ctx.enter_context(tc.tile_pool(name="singles", bufs=1))
    xpool = ctx.enter_context(tc.tile_pool(name="xpool", bufs=12))
    pinpool = ctx.enter_context(tc.tile_pool(name="pinpool", bufs=2))
    scratch = ctx.enter_context(tc.tile_pool(name="scratch", bufs=2))
    spool = ctx.enter_context(tc.tile_pool(name="spool", bufs=4))
    hpool = ctx.enter_context(tc.tile_pool(name="hpool", bufs=4))
    gpool = ctx.enter_context(tc.tile_pool(name="gpool", bufs=4))
    pp_h = ctx.enter_context(tc.tile_pool(name="pp_h", bufs=2, space="PSUM"))
    pp_g = ctx.enter_context(tc.tile_pool(name="pp_g", bufs=2, space="PSUM"))

    def reduce_half(s, hh, xt):
        # half hh covers blocks f = 2*hh, 2*hh+1 (channel 4p+f on
        # partition p). One block on DVE, one on ACT so they overlap;
        # ACT copies into scratch so the x tile is only READ here.
        # (Alternatives measured and rejected: DVE scalar_tensor_tensor
        # fold with accum_out is ~5x cheaper on paper but showed
        # run-to-run numerical wobble right at the 2e-2 gate; GpSimd
        # tensor_reduce only supports partition-axis reduction.)
        f0, f1 = 2 * hh, 2 * hh + 1
        nc.vector.reduce_sum(s[:, f0:f0 + 1], xt[:, 0:HW],
                             axis=mybir.AxisListType.X)
        tr = scratch.tile([P, HW], bf16, tag="trash")
        nc.scalar.activation(tr, xt[:, HW:2 * HW],
                             mybir.ActivationFunctionType.Copy,
                             accum_out=s[:, f1:f1 + 1])

    # ---- sample 0 loads first so HBM streaming starts immediately ----
    # split into 32-partition sub-transfers: queue doorbells ring per
    # (transfer x queue) batch, so small leading transfers engage all 16
    # queues ~5us earlier than one 128-line transfer would
    s0 = spool.tile([P, F], f32)
    x0a = pinpool.tile([P, HHW], bf16, tag="pin")
    for k in range(0, P, 32):
        nc.sync.dma_start(out=x0a[k:k + 32, :], in_=x[0, k:k + 32, 0:HHW])
    x0b = pinpool.tile([P, HHW], bf16, tag="pin")
    for k in range(0, P, 64):
        nc.sync.dma_start(out=x0b[k:k + 64, :], in_=x[0, k:k + 64, HHW:FHW])

    # ---- weights (host-prepped layouts) ride the idle Scalar ring ----
    w1s = singles.tile([P, F, R], f32)               # lhsT for h-matmul, /HW folded
    nc.scalar.dma_start(out=w1s, in_=w1t)
    w2s = singles.tile([R, F, P], f32)               # lhsT for g-matmul
    nc.scalar.dma_start(out=w2s, in_=w2t)
    b1s = singles.tile([R, 1], f32)
    nc.scalar.dma_start(out=b1s, in_=b1.rearrange("(r o) -> r o", o=1))
    b2s = singles.tile([P, F], f32)
    nc.scalar.dma_start(out=b2s, in_=b2t)

    def gate(s):
        # h = relu(w1 @ mean + b1): accumulate over the 4 channel blocks
        ph = pp_h.tile([R, 1], f32)
        for f in range(F):
            nc.tensor.matmul(ph, w1s[:, f, :], s[:, f:f + 1],
                             start=(f == 0), stop=(f == F - 1))
        h = hpool.tile([R, 1], f32)
        nc.vector.tensor_scalar(out=h, in0=ph, scalar1=b1s, scalar2=0.0,
                                op0=mybir.AluOpType.add, op1=mybir.AluOpType.max)
        # g[:, f] = sigmoid(w2[4p+f] @ h + b2[4p+f])
        pg = pp_g.tile([P, F], f32)
        g = gpool.tile([P, F], f32)
        for f in range(F):
            nc.tensor.matmul(pg[:, f:f + 1], w2s[:, f, :], h, start=True, stop=True)
            nc.scalar.activation(g[:, f:f + 1], pg[:, f:f + 1],
                                 mybir.ActivationFunctionType.Sigmoid,
                                 bias=b2s[:, f:f + 1], scale=1.0)
        return g

    def scale_half(xt, g, hh):
        # DVE tensor_scalar in bf16 hits the 4x perf mode (~0.95us/block)
        f0, f1 = 2 * hh, 2 * hh + 1
        nc.vector.tensor_scalar_mul(xt[:, 0:HW], xt[:, 0:HW], g[:, f0:f0 + 1])
        nc.vector.tensor_scalar_mul(xt[:, HW:2 * HW], xt[:, HW:2 * HW],
                                    g[:, f1:f1 + 1])

    def store_half(b, hh, xt):
        nc.scalar.dma_start(out=out[b, :, hh * HHW:(hh + 1) * HHW], in_=xt)

    for hh, xt in ((0, x0a), (1, x0b)):
        reduce_half(s0, hh, xt)
    g0 = gate(s0)
    scale_half(x0a, g0, 0)
    scale_half(x0b, g0, 1)
    # sample 0's stores are deferred to the drain window (see below)

    for b in range(1, BL):
        s = spool.tile([P, F], f32)
        xts = []
        for hh in range(2):
            xt = xpool.tile([P, HHW], bf16, tag="x")
            nc.sync.dma_start(out=xt, in_=x[b, :, hh * HHW:(hh + 1) * HHW])
            reduce_half(s, hh, xt)
            xts.append(xt)
        if b == BL - 1:
            # sample 0's (long-ready) stores go ahead of the last
            # sample's, keeping DMA busy during its gate latency
            store_half(0, 0, x0a)
            store_half(0, 1, x0b)
        g = gate(s)
        for hh in range(2):
            scale_half(xts[hh], g, hh)
            store_half(b, hh, xts[hh])


def _build():
    f32 = mybir.dt.float32
    bf16 = mybir.dt.bfloat16
    nc = bacc.Bacc("TRN2", target_bir_lowering=False, debug=False,
                   num_devices=N_CORES)
    # x/out are the same bytes as [BL, C, HW]: partition p <-> channels
    # 4p..4p+3, contiguous in DRAM, so each partition line is contiguous.
    x = nc.dram_tensor("x", [BL, P, FHW], bf16, kind="ExternalInput").ap()
    w1t = nc.dram_tensor("w1t", [P, F, R], f32, kind="ExternalInput").ap()
    b1 = nc.dram_tensor("b1", [R], f32, kind="ExternalInput").ap()
    w2t = nc.dram_tensor("w2t", [R, F, P], f32, kind="ExternalInput").ap()
    b2t = nc.dram_tensor("b2t", [P, F], f32, kind="ExternalInput").ap()
    out = nc.dram_tensor("out", [BL, P, FHW], bf16, kind="ExternalOutput").ap()

    with tile.TileContext(nc) as tc:
        with contextlib.ExitStack() as ctx:
            _emit(ctx, tc, out, x, w1t, b1, w2t, b2t)
    nc.compile()
    return nc


def _get_module():
    if "nc" not in _CACHE:
        _CACHE["nc"] = _build()
    return _CACHE["nc"]


def kernel(**inputs):
    global LAST_RESULTS
    bf16_np = mybir.dt.np(mybir.dt.bfloat16)
    x = np.ascontiguousarray(
        np.asarray(inputs["x"], dtype=np.float32).astype(bf16_np))
    w1 = np.asarray(inputs["w1"], dtype=np.float32)
    b1 = np.ascontiguousarray(inputs["b1"], dtype=np.float32)
    w2 = np.asarray(inputs["w2"], dtype=np.float32)
    b2 = np.asarray(inputs["b2"], dtype=np.float32)

    # host-side prep: matmul-ready weight layouts for the channel mapping
    # c = 4p+f (tiny tensors)
    # w1t[p, f, r] = w1[r, 4p+f] / HW   (lhsT for the h-matmul)
    w1t = np.ascontiguousarray((w1.T * INV_HW).reshape(P, F, R))
    # w2t[r, f, p] = w2[4p+f, r]        (lhsT for the g-matmul)
    w2t = np.ascontiguousarray(w2.reshape(P, F, R).transpose(2, 1, 0))
    # b2t[p, f] = b2[4p+f]
    b2t = np.ascontiguousarray(b2.reshape(P, F))

    nc = _get_module()
    xr = x.reshape(B, P, FHW)
    in_maps = [
        {
            "x": xr[i * BL:(i + 1) * BL],
            "w1t": w1t,
            "b1": b1,
            "w2t": w2t,
            "b2t": b2t,
        }
        for i in range(N_CORES)
    ]
    res = bass_utils.run_bass_kernel_spmd(
        nc, in_maps, core_ids=list(range(N_CORES))
    )
    LAST_RESULTS = res
    out = np.concatenate(
        [np.asarray(res.results[i]["out"]) for i in range(N_CORES)], axis=0)
    return out.astype(np.float32).reshape(B, C, H, W)
